# revision 7
# baseline (speedup 1.0000x reference)
"""Multi-head causal attention (B=4, S=2048, D=1024, H=16) on 8 Trainium2
NeuronCores via Bass/Tile.

Sharding: core c handles batch b = c//2 and head-group g = c%2 (8 heads,
i.e. columns [512g, 512g+512) of Wq/Wk/Wv and rows [512g, 512g+512) of Wo).
Each core computes its 8 heads' attention and a partial output projection
[S, D]; the host sums the two head-group partials per batch and adds bo.

All matmuls run in float32r (full-rate fp32 mode of the PE, ~1.5e-4 rel
err); softmax runs in fp32 on ACT/DVE. Layout choices keep every matmul at
N=512 moving columns:
  qT/kT:  [dk, s]  (projection emitted transposed: lhsT=W chunk, rhs=X^T)
  v:      [s, dk]  interleaved with a ones column per head ([..v_h.., 1])
          so the attention-V matmul also produces the softmax row-sums
  scores: [sk, sq] (transposed; lhsT=kT chunk, rhs=qT) -> exp -> expT
  AV:     av[65, sq] += v_aug^T @ expT  (row 64 = softmax denominators)
  out:    partial[sq, :] = ctx^T.T @ Wo  (ctx^T is exactly the AV output)
"""

import sys
import numpy as np

for _p in ("/opt/trn_rl_repo", "/root/.axon_site/_ro/trn_rl_repo"):
    if _p not in sys.path:
        sys.path.append(_p)

B, S_FULL, D, H, DK = 4, 2048, 1024, 16, 64
GD = 512          # dk span per core (8 heads)
P = 128
NPAIR = GD // P   # 4 head-pairs per core
N_CORES = 8
MASK_NEG = -8.0e9  # multiplied by the 0.125 softmax scale inside exp -> -1e9

_BUILD_CACHE = {}


def _build(s_len, causal):
    from contextlib import ExitStack

    import concourse.tile as tile
    from concourse import bacc, mybir

    dt = mybir.dt
    f32, f32r, bf16 = dt.float32, dt.float32r, dt.bfloat16
    Exp = mybir.ActivationFunctionType.Exp

    S = s_len
    SJ = S // 512     # 512-wide sq chunks
    SM = S // P       # 128-wide sk chunks
    DC = D // P       # contraction chunks for the projections

    nc = bacc.Bacc("TRN2", target_bir_lowering=False, debug=False,
                   num_devices=N_CORES)

    xq = nc.dram_tensor("xq", [D, S], f32r, kind="ExternalInput")
    xk = nc.dram_tensor("xk", [D, S], f32r, kind="ExternalInput")
    xv = nc.dram_tensor("xv", [D, S], f32r, kind="ExternalInput")
    wq = nc.dram_tensor("wq", [D, GD], f32r, kind="ExternalInput")
    wk = nc.dram_tensor("wk", [D, GD], f32r, kind="ExternalInput")
    wv = nc.dram_tensor("wv", [D, GD], f32r, kind="ExternalInput")
    wo = nc.dram_tensor("wo", [GD, D], f32r, kind="ExternalInput")
    bq = nc.dram_tensor("bq", [1, GD], f32r, kind="ExternalInput")
    bk = nc.dram_tensor("bk", [1, GD], f32r, kind="ExternalInput")
    bv = nc.dram_tensor("bv", [1, GD], f32r, kind="ExternalInput")
    ones_row = nc.dram_tensor("ones_row", [1, 512], f32r, kind="ExternalInput")
    ones_col = nc.dram_tensor("ones_col", [1, P], f32r, kind="ExternalInput")
    ones_vcol = nc.dram_tensor("ones_vcol", [P, 8], f32r, kind="ExternalInput")
    if causal:
        # 4 canonical diagonal-band blocks: block d, entry [p, c] masked
        # when p + 128*d > c  (value MASK_NEG, else 0)
        maskd = nc.dram_tensor("maskd", [4 * P, 512], bf16, kind="ExternalInput")
    else:
        # full transposed mask [sk, sq] * MASK_NEG
        maskt = nc.dram_tensor("maskt", [S, S], bf16, kind="ExternalInput")
    out = nc.dram_tensor("out", [S, D], f32, kind="ExternalOutput")

    with tile.TileContext(nc) as tc, ExitStack() as ctx0:
        persist = ctx0.enter_context(tc.tile_pool(name="persist", bufs=1))

        qTt = [persist.tile([P, S], f32r, tag=f"qT{p}", name=f"qT{p}") for p in range(NPAIR)]
        kTt = [persist.tile([P, S], f32r, tag=f"kT{p}", name=f"kT{p}") for p in range(NPAIR)]
        vt = [persist.tile([P, 8 * 65], f32r, tag=f"v{m}", name=f"v{m}") for m in range(SM)]
        ones_row_t = persist.tile([1, 512], f32r, tag="ones_row")
        ones_col_t = persist.tile([1, P], f32r, tag="ones_col")
        bq_t = persist.tile([1, GD], f32r, tag="bq")
        bk_t = persist.tile([1, GD], f32r, tag="bk")
        bv_t = persist.tile([1, GD], f32r, tag="bv")
        nc.sync.dma_start(ones_row_t[:], ones_row.ap())
        nc.sync.dma_start(ones_col_t[:], ones_col.ap())
        nc.sync.dma_start(bq_t[:], bq.ap())
        nc.sync.dma_start(bk_t[:], bk.ap())
        nc.sync.dma_start(bv_t[:], bv.ap())
        if causal:
            maskd_t = [persist.tile([P, 512], bf16, tag=f"maskd{d}", name=f"maskd{d}")
                       for d in range(4)]
            for d in range(4):
                nc.sync.dma_start(maskd_t[d][:], maskd.ap()[d * P:(d + 1) * P, :])

        # ---------------- phase 1: projections ----------------
        # X^T is staged in [128, 512] column slices (8 contraction chunks
        # live per sq-chunk) to stay inside the SBUF budget.
        with ExitStack() as ctx1:
            xpool = ctx1.enter_context(tc.tile_pool(name="xt", bufs=18))
            wpool = ctx1.enter_context(tc.tile_pool(name="wt", bufs=10))
            ps1 = ctx1.enter_context(tc.tile_pool(name="ps1", bufs=3, space="PSUM"))

            # q and k projections, emitted transposed [dk, s]
            for x_d, w_d, b_t, outT in ((xq, wq, bq_t, qTt), (xk, wk, bk_t, kTt)):
                wt = []
                for dc in range(DC):
                    w = wpool.tile([P, GD], f32r, tag="wt", name="wt")
                    nc.sync.dma_start(w[:], w_d.ap()[dc * P:(dc + 1) * P, :])
                    wt.append(w)
                for sj in range(SJ):
                    xt = []
                    for dc in range(DC):
                        t = xpool.tile([P, 512], f32r, tag="xt", name="xt")
                        nc.sync.dma_start(
                            t[:], x_d.ap()[dc * P:(dc + 1) * P,
                                           sj * 512:(sj + 1) * 512])
                        xt.append(t)
                    for i in range(NPAIR):
                        ps = ps1.tile([P, 512], f32, tag="ps1", name="ps1")
                        for dc in range(DC):
                            nc.tensor.matmul(
                                ps[:],
                                wt[dc][:, i * P:(i + 1) * P],
                                xt[dc][:],
                                start=(dc == 0), stop=False)
                        nc.tensor.matmul(
                            ps[:], b_t[0:1, i * P:(i + 1) * P], ones_row_t[:],
                            start=False, stop=True)
                        nc.vector.tensor_copy(
                            outT[i][:, sj * 512:(sj + 1) * 512], ps[:])

            # v projection, natural layout [s, dk] with interleaved ones col
            wt = []
            for dc in range(DC):
                w = wpool.tile([P, GD], f32r, tag="wt", name="wt")
                nc.sync.dma_start(w[:], wv.ap()[dc * P:(dc + 1) * P, :])
                wt.append(w)
            for sj in range(SJ):
                xt = []
                for dc in range(DC):
                    t = xpool.tile([P, 512], f32r, tag="xt", name="xt")
                    nc.sync.dma_start(
                        t[:], xv.ap()[dc * P:(dc + 1) * P,
                                      sj * 512:(sj + 1) * 512])
                    xt.append(t)
                for si in range(4 * sj, 4 * sj + 4):
                    c0 = (si - 4 * sj) * P
                    ps = ps1.tile([P, 512], f32, tag="ps1", name="ps1")
                    for dc in range(DC):
                        nc.tensor.matmul(
                            ps[:],
                            xt[dc][:, c0:c0 + P],
                            wt[dc][:],
                            start=(dc == 0), stop=False)
                    nc.tensor.matmul(ps[:], ones_col_t[:], bv_t[:],
                                     start=False, stop=True)
                    v3 = vt[si][:].rearrange("p (h c) -> p h c", h=8)
                    nc.vector.tensor_copy(
                        v3[:, :, 0:64],
                        ps[:].rearrange("p (h c) -> p h c", h=8))
                    nc.sync.dma_start(v3[:, :, 64:65], ones_vcol.ap()[:, :, None])

        # ---------------- phases 2+3: attention + output projection ------
        with ExitStack() as ctx2:
            ctxpool = ctx2.enter_context(tc.tile_pool(name="ctxp", bufs=1))
            ctxt = [ctxpool.tile([P, S], f32r, tag=f"ctx{p}", name=f"ctx{p}")
                    for p in range(NPAIR)]
            wopool = ctx2.enter_context(tc.tile_pool(name="wo", bufs=1))
            scps = ctx2.enter_context(tc.tile_pool(name="scps", bufs=2, space="PSUM"))
            avps = ctx2.enter_context(tc.tile_pool(name="avps", bufs=2, space="PSUM"))
            ps3 = ctx2.enter_context(tc.tile_pool(name="ps3", bufs=2, space="PSUM"))
            nb = 3 if causal else 2  # general variant needs room for mpool
            expp = ctx2.enter_context(tc.tile_pool(name="expp", bufs=nb))
            avsb = ctx2.enter_context(tc.tile_pool(name="avsb", bufs=2))
            rcp = ctx2.enter_context(tc.tile_pool(name="rcp", bufs=2))
            osb = ctx2.enter_context(tc.tile_pool(name="osb", bufs=nb))
            if not causal:
                mpool = ctx2.enter_context(tc.tile_pool(name="mp", bufs=SM + 2))

            wot = [[wopool.tile([P, 512], f32r, tag=f"wo{p}_{h}", name=f"wo{p}_{h}")
                    for h in range(D // 512)] for p in range(NPAIR)]
            for p in range(NPAIR):
                for h in range(D // 512):
                    nc.sync.dma_start(
                        wot[p][h][:],
                        wo.ap()[p * P:(p + 1) * P, h * 512:(h + 1) * 512])

            for j in range(SJ):
                n_m = 4 * (j + 1) if causal else SM
                if not causal:
                    mt = []
                    for m in range(SM):
                        t = mpool.tile([P, 512], bf16, tag="mt", name="mt")
                        nc.sync.dma_start(
                            t[:], maskt.ap()[m * P:(m + 1) * P,
                                             j * 512:(j + 1) * 512])
                        mt.append(t)
                for p in range(NPAIR):
                    av = [avps.tile([65, 512], f32, tag="av", name="av") for _ in range(2)]
                    pend = None  # (m, exp_tile) awaiting its AV matmuls
                    for m in range(n_m):
                        sc = scps.tile([P, 1024], f32, tag="sc", name="sc")
                        for e in range(2):
                            nc.tensor.matmul(
                                sc[:, e * 512:(e + 1) * 512],
                                kTt[p][64 * e:64 * e + 64, m * P:(m + 1) * P],
                                qTt[p][64 * e:64 * e + 64, j * 512:(j + 1) * 512],
                                start=True, stop=True)
                        sc3 = sc[:].rearrange("p (e c) -> p e c", e=2)
                        if causal:
                            d = m - 4 * j
                            if d >= 0:
                                nc.vector.tensor_add(
                                    sc3, sc3,
                                    maskd_t[d][:][:, None, :].broadcast_to(
                                        [P, 2, 512]))
                        else:
                            nc.vector.tensor_add(
                                sc3, sc3,
                                mt[m][:][:, None, :].broadcast_to([P, 2, 512]))
                        ex = expp.tile([P, 1024], f32r, tag="ex", name="ex")
                        nc.scalar.activation(ex[:], sc[:], Exp, scale=0.125)
                        if pend is not None:
                            pm, pex = pend
                            for e in range(2):
                                nc.tensor.matmul(
                                    av[e][:],
                                    vt[pm][:, 65 * (2 * p + e):65 * (2 * p + e) + 65],
                                    pex[:, e * 512:(e + 1) * 512],
                                    start=(pm == 0), stop=(pm == n_m - 1))
                        pend = (m, ex)
                    pm, pex = pend
                    for e in range(2):
                        nc.tensor.matmul(
                            av[e][:],
                            vt[pm][:, 65 * (2 * p + e):65 * (2 * p + e) + 65],
                            pex[:, e * 512:(e + 1) * 512],
                            start=(pm == 0), stop=(pm == n_m - 1))
                    # normalize: ctx^T[dk, sq] = av[0:64] / av[64]
                    for e in range(2):
                        asb = avsb.tile([65, 512], f32, tag="asb", name="asb")
                        nc.vector.tensor_copy(asb[:], av[e][:])
                        r = rcp.tile([1, 512], f32r, tag="r", name="r")
                        with nc.allow_low_precision(
                                reason="softmax denominators are O(1..1e3); "
                                       "f32r reciprocal keeps ~1e-4 rel err"):
                            nc.vector.reciprocal(r[:], asb[64:65, :])
                        bc = avps.tile([65, 512], f32, tag="av", name="bc")
                        nc.tensor.matmul(bc[0:64, :], ones_col_t[0:1, 0:64],
                                         r[:], start=True, stop=True)
                        nc.vector.tensor_mul(
                            ctxt[p][64 * e:64 * e + 64, j * 512:(j + 1) * 512],
                            asb[0:64, :], bc[0:64, :])

                # output projection for the sq rows finished in this j chunk
                for si in range(4 * j, 4 * j + 4) if S >= 512 else range(SM):
                    ot = osb.tile([P, D], f32, tag="ot", name="ot")
                    for h in range(D // 512):
                        ps = ps3.tile([P, 512], f32, tag="ps3", name="ps3")
                        for p in range(NPAIR):
                            nc.tensor.matmul(
                                ps[:],
                                ctxt[p][:, si * P:(si + 1) * P],
                                wot[p][h][:],
                                start=(p == 0), stop=(p == NPAIR - 1))
                        nc.vector.tensor_copy(ot[:, h * 512:(h + 1) * 512], ps[:])
                    nc.sync.dma_start(out.ap()[si * P:(si + 1) * P, :], ot[:])

    nc.compile()
    return nc


def _get_nc(s_len, causal):
    key = (s_len, causal)
    if key not in _BUILD_CACHE:
        _BUILD_CACHE[key] = _build(s_len, causal)
    return _BUILD_CACHE[key]


def kernel(query, key, value, mask, Wq, bq, Wk, bk, Wv, bv, Wo, bo):
    import ml_dtypes
    from concourse.bass_utils import run_bass_kernel_spmd

    query = np.asarray(query, dtype=np.float32)
    key = np.asarray(key, dtype=np.float32)
    value = np.asarray(value, dtype=np.float32)
    mask = np.asarray(mask, dtype=np.float32)
    Wq, Wk, Wv, Wo = (np.asarray(w, dtype=np.float32) for w in (Wq, Wk, Wv, Wo))
    bq, bk, bv, bo = (np.asarray(b, dtype=np.float32) for b in (bq, bk, bv, bo))

    b_sz, s_len, d = query.shape
    m2 = mask.reshape(s_len, s_len)
    causal = bool(
        np.array_equal(m2, np.triu(np.ones((s_len, s_len), np.float32), k=1)))

    nc = _get_nc(s_len, causal)

    ones_row = np.ones((1, 512), np.float32)
    ones_col = np.ones((1, P), np.float32)
    ones_vcol = np.ones((P, 8), np.float32)
    if causal:
        # maskd[d][p, c] = MASK_NEG where p + 128*d > c
        pp = np.arange(P)[:, None]
        cc = np.arange(512)[None, :]
        maskd = np.concatenate(
            [np.where(pp + P * dd > cc, MASK_NEG, 0.0) for dd in range(4)],
            axis=0).astype(ml_dtypes.bfloat16)
    else:
        maskt = (m2.T * MASK_NEG).astype(ml_dtypes.bfloat16)

    in_maps = []
    for c in range(N_CORES):
        b = c // 2
        g = c % 2
        cols = slice(GD * g, GD * g + GD)
        im = {
            "xq": np.ascontiguousarray(query[b].T),
            "xk": np.ascontiguousarray(key[b].T),
            "xv": np.ascontiguousarray(value[b].T),
            "wq": np.ascontiguousarray(Wq[:, cols]),
            "wk": np.ascontiguousarray(Wk[:, cols]),
            "wv": np.ascontiguousarray(Wv[:, cols]),
            "wo": np.ascontiguousarray(Wo[cols, :]),
            "bq": bq[cols].reshape(1, GD),
            "bk": bk[cols].reshape(1, GD),
            "bv": bv[cols].reshape(1, GD),
            "ones_row": ones_row,
            "ones_col": ones_col,
            "ones_vcol": ones_vcol,
        }
        if causal:
            im["maskd"] = maskd
        else:
            im["maskt"] = maskt
        in_maps.append(im)

    res = run_bass_kernel_spmd(nc, in_maps, list(range(N_CORES)))

    out = np.empty((b_sz, s_len, d), np.float32)
    for b in range(b_sz):
        out[b] = res.results[2 * b]["out"] + res.results[2 * b + 1]["out"] + bo
    return out


# revision 24
# speedup vs baseline: 1.4079x; 1.4079x over previous
"""Multi-head causal attention (B=4, S=2048, D=1024, H=16) on 8 Trainium2
NeuronCores via Bass/Tile.

Sharding: core c handles batch b = c//2 and head-group g = c%2 (8 heads,
i.e. columns [512g, 512g+512) of Wq/Wk/Wv and rows [512g, 512g+512) of Wo).
Each core computes its 8 heads' attention and a partial output projection
[S, D]; the host sums the two head-group partials per batch and adds bo.

Matmul operands are fp16 (full-rate 1 cycle/row on the PE; fp32 accumulate
in PSUM); softmax runs in fp32 on ACT/DVE. All values are O(100) or less so
fp16 range is safe, and fp16's 10-bit mantissa keeps the end-to-end error
around 5e-4. Layouts keep every matmul at N=512 moving columns:
  qT/kT:  [dk, s]  (projection emitted transposed: lhsT=W chunk, rhs=X^T)
  v:      [s, dk]  interleaved with a ones column per head ([..v_h.., 1])
          so the attention-V matmul also produces the softmax row-sums
  scores: [sk, sq] (transposed; lhsT=kT chunk, rhs=qT) -> exp -> expT
  AV:     av[65, sq] += v_aug^T @ expT  (row 64 = softmax denominators)
  out:    partial[sq, :] = ctx^T.T @ Wo  (ctx^T is exactly the AV output)
"""

import os
import sys
import numpy as np

for _p in ("/opt/trn_rl_repo", "/root/.axon_site/_ro/trn_rl_repo"):
    if _p not in sys.path:
        sys.path.append(_p)

B, S_FULL, D, H, DK = 4, 2048, 1024, 16, 64
GD = 512          # dk span per core (8 heads)
P = 128
NPAIR = GD // P   # 4 head-pairs per core
N_CORES = 8
MASK_NEG = -8.0e9  # multiplied by the 0.125 softmax scale inside exp -> -1e9

_BUILD_CACHE = {}


def _build(s_len, causal):
    from contextlib import ExitStack

    import concourse.tile as tile
    from concourse import bacc, mybir

    dt = mybir.dt
    f32, f16, bf16 = dt.float32, dt.float16, dt.bfloat16
    Exp = mybir.ActivationFunctionType.Exp

    S = s_len
    SJ = S // 512     # 512-wide sq chunks
    SM = S // P       # 128-wide sk chunks
    DC = D // P       # contraction chunks for the projections

    nc = bacc.Bacc("TRN2", target_bir_lowering=False, debug=False,
                   num_devices=N_CORES)

    xq = nc.dram_tensor("xq", [D, S], f16, kind="ExternalInput")
    xk = nc.dram_tensor("xk", [D, S], f16, kind="ExternalInput")
    xv = nc.dram_tensor("xv", [D, S], f16, kind="ExternalInput")
    wq = nc.dram_tensor("wq", [D, GD], f16, kind="ExternalInput")
    wk = nc.dram_tensor("wk", [D, GD], f16, kind="ExternalInput")
    wv = nc.dram_tensor("wv", [D, GD], f16, kind="ExternalInput")
    wo = nc.dram_tensor("wo", [GD, D], f16, kind="ExternalInput")
    bq = nc.dram_tensor("bq", [1, GD], f16, kind="ExternalInput")
    bk = nc.dram_tensor("bk", [1, GD], f16, kind="ExternalInput")
    bv = nc.dram_tensor("bv", [1, GD], f16, kind="ExternalInput")
    ones_row = nc.dram_tensor("ones_row", [1, 512], f16, kind="ExternalInput")
    ones_col = nc.dram_tensor("ones_col", [1, P], f16, kind="ExternalInput")
    ones_vcol = nc.dram_tensor("ones_vcol", [P, 8], f16, kind="ExternalInput")
    # selh[:, 64r:64r+64] is row-r-one-hot: selects head r's reciprocal row
    # and broadcasts it over 64 partitions in one K=8 matmul
    selh = nc.dram_tensor("selh", [8, 8 * 64], f16, kind="ExternalInput")
    # oneh8[0, 8r:8r+8] is the one-hot row e_r: routes head r's softmax
    # denominator row into partition r of the gathered [8, 512] psum tile
    oneh8 = nc.dram_tensor("oneh8", [1, 64], f16, kind="ExternalInput")
    if causal:
        # 4 canonical diagonal-band blocks: block d, entry [p, c] masked
        # when p + 128*d > c  (value MASK_NEG, else 0)
        maskd = nc.dram_tensor("maskd", [4 * P, 512], bf16, kind="ExternalInput")
    else:
        # full transposed mask [sk, sq] * MASK_NEG
        maskt = nc.dram_tensor("maskt", [S, S], bf16, kind="ExternalInput")
    out = nc.dram_tensor("out", [S, D], f32, kind="ExternalOutput")

    with tile.TileContext(nc) as tc, ExitStack() as ctx0:
        persist = ctx0.enter_context(tc.tile_pool(name="persist", bufs=1))

        qTt = [persist.tile([P, S], f16, tag=f"qT{p}", name=f"qT{p}")
               for p in range(NPAIR)]
        kTt = [persist.tile([P, S], f16, tag=f"kT{p}", name=f"kT{p}")
               for p in range(NPAIR)]
        vt = [persist.tile([P, 8 * 65], f16, tag=f"v{m}", name=f"v{m}")
              for m in range(SM)]
        ones_row_t = persist.tile([1, 512], f16, tag="ones_row")
        ones_col_t = persist.tile([1, P], f16, tag="ones_col")
        bq_t = persist.tile([1, GD], f16, tag="bq")
        bk_t = persist.tile([1, GD], f16, tag="bk")
        bv_t = persist.tile([1, GD], f16, tag="bv")
        selh_t = persist.tile([8, 8 * 64], f16, tag="selh")
        # staged at partition 64 so its base matches asb[64:65] in the
        # denominator-gather matmul (lhsT/rhs bases must agree)
        oneh8_t = persist.tile([65, 64], f16, tag="oneh8")
        nc.sync.dma_start(selh_t[:], selh.ap())
        nc.sync.dma_start(oneh8_t[64:65, :], oneh8.ap())
        nc.sync.dma_start(ones_row_t[:], ones_row.ap())
        nc.sync.dma_start(ones_col_t[:], ones_col.ap())
        nc.sync.dma_start(bq_t[:], bq.ap())
        nc.sync.dma_start(bk_t[:], bk.ap())
        nc.sync.dma_start(bv_t[:], bv.ap())
        if causal:
            maskd_t = [persist.tile([P, 512], bf16, tag=f"maskd{d}",
                                    name=f"maskd{d}")
                       for d in range(4)]
            for d in range(4):
                nc.sync.dma_start(maskd_t[d][:], maskd.ap()[d * P:(d + 1) * P, :])

        # ---------------- phase 1: projections ----------------
        # X^T is staged in [128, 512] column slices (8 contraction chunks
        # live per sq-chunk) to stay inside the SBUF budget.
        with ExitStack() as ctx1:
            xpool = ctx1.enter_context(tc.tile_pool(name="xt", bufs=20))
            wpool = ctx1.enter_context(tc.tile_pool(name="wt", bufs=10))
            ps1 = ctx1.enter_context(tc.tile_pool(name="ps1", bufs=3, space="PSUM"))

            # q and k projections, emitted transposed [dk, s]
            for x_d, w_d, b_t, outT in ((xq, wq, bq_t, qTt), (xk, wk, bk_t, kTt)):
                wt = []
                for dc in range(DC):
                    w = wpool.tile([P, GD], f16, tag="wt", name="wt")
                    nc.sync.dma_start(w[:], w_d.ap()[dc * P:(dc + 1) * P, :])
                    wt.append(w)
                for sj in range(SJ):
                    xt = []
                    for dc in range(DC):
                        t = xpool.tile([P, 512], f16, tag="xt", name="xt")
                        nc.sync.dma_start(
                            t[:], x_d.ap()[dc * P:(dc + 1) * P,
                                           sj * 512:(sj + 1) * 512])
                        xt.append(t)
                    for i in range(NPAIR):
                        ps = ps1.tile([P, 512], f32, tag="ps1", name="ps1")
                        for dc in range(DC):
                            nc.tensor.matmul(
                                ps[:],
                                wt[dc][:, i * P:(i + 1) * P],
                                xt[dc][:],
                                start=(dc == 0), stop=False)
                        nc.tensor.matmul(
                            ps[:], b_t[0:1, i * P:(i + 1) * P], ones_row_t[:],
                            start=False, stop=True)
                        nc.vector.tensor_copy(
                            outT[i][:, sj * 512:(sj + 1) * 512], ps[:])

            # v projection, natural layout [s, dk] with interleaved ones col
            wt = []
            for dc in range(DC):
                w = wpool.tile([P, GD], f16, tag="wt", name="wt")
                nc.sync.dma_start(w[:], wv.ap()[dc * P:(dc + 1) * P, :])
                wt.append(w)
            for sj in range(SJ):
                xt = []
                for dc in range(DC):
                    t = xpool.tile([P, 512], f16, tag="xt", name="xt")
                    nc.sync.dma_start(
                        t[:], xv.ap()[dc * P:(dc + 1) * P,
                                      sj * 512:(sj + 1) * 512])
                    xt.append(t)
                for si in range(4 * sj, 4 * sj + 4):
                    c0 = (si - 4 * sj) * P
                    ps = ps1.tile([P, 512], f32, tag="ps1", name="ps1")
                    for dc in range(DC):
                        nc.tensor.matmul(
                            ps[:],
                            xt[dc][:, c0:c0 + P],
                            wt[dc][:],
                            start=(dc == 0), stop=False)
                    nc.tensor.matmul(ps[:], ones_col_t[:], bv_t[:],
                                     start=False, stop=True)
                    v3 = vt[si][:].rearrange("p (h c) -> p h c", h=8)
                    nc.vector.tensor_copy(
                        v3[:, :, 0:64],
                        ps[:].rearrange("p (h c) -> p h c", h=8))
                    nc.sync.dma_start(v3[:, :, 64:65], ones_vcol.ap()[:, :, None])

        # ---------------- phases 2+3: attention + output projection ------
        with ExitStack() as ctx2:
            ctxpool = ctx2.enter_context(tc.tile_pool(name="ctxp", bufs=1))
            ctxt = [ctxpool.tile([P, S], f16, tag=f"ctx{p}", name=f"ctx{p}")
                    for p in range(NPAIR)]
            wopool = ctx2.enter_context(tc.tile_pool(name="wo", bufs=1))
            scps = ctx2.enter_context(tc.tile_pool(name="scps", bufs=2, space="PSUM"))
            avps = ctx2.enter_context(tc.tile_pool(name="avps", bufs=4, space="PSUM"))
            ps3 = avps
            expp = ctx2.enter_context(tc.tile_pool(name="expp", bufs=4))
            avsb = ctx2.enter_context(tc.tile_pool(name="avsb", bufs=10))
            rcp = ctx2.enter_context(tc.tile_pool(name="rcp", bufs=2))
            osb = ctx2.enter_context(tc.tile_pool(name="osb", bufs=3))
            if not causal:
                mpool = ctx2.enter_context(tc.tile_pool(name="mp", bufs=SM + 2))

            wot = [[wopool.tile([P, 512], f16, tag=f"wo{p}_{h}", name=f"wo{p}_{h}")
                    for h in range(D // 512)] for p in range(NPAIR)]
            for p in range(NPAIR):
                for h in range(D // 512):
                    nc.sync.dma_start(
                        wot[p][h][:],
                        wo.ap()[p * P:(p + 1) * P, h * 512:(h + 1) * 512])

            for j in range(SJ):
                n_m = 4 * (j + 1) if causal else SM
                if not causal:
                    mt = []
                    for m in range(SM):
                        t = mpool.tile([P, 512], bf16, tag="mt", name="mt")
                        nc.sync.dma_start(
                            t[:], maskt.ap()[m * P:(m + 1) * P,
                                             j * 512:(j + 1) * 512])
                        mt.append(t)
                den = avps.tile([8, 512], f32, tag="av", name="den")
                asb_all = {}
                for p in range(NPAIR):
                    av = [avps.tile([65, 512], f32, tag="av", name="av")
                          for _ in range(2)]
                    pend = None  # (m, exp_tile) awaiting its AV matmuls
                    for m in range(n_m):
                        sc = scps.tile([P, 1024], f32, tag="sc", name="sc")
                        for e in range(2):
                            nc.tensor.matmul(
                                sc[:, e * 512:(e + 1) * 512],
                                kTt[p][64 * e:64 * e + 64, m * P:(m + 1) * P],
                                qTt[p][64 * e:64 * e + 64, j * 512:(j + 1) * 512],
                                start=True, stop=True)
                        sc3 = sc[:].rearrange("p (e c) -> p e c", e=2)
                        if causal:
                            d = m - 4 * j
                            if d >= 0:
                                nc.vector.tensor_add(
                                    sc3, sc3,
                                    maskd_t[d][:][:, None, :].broadcast_to(
                                        [P, 2, 512]))
                        else:
                            nc.vector.tensor_add(
                                sc3, sc3,
                                mt[m][:][:, None, :].broadcast_to([P, 2, 512]))
                        ex = expp.tile([P, 1024], f16, tag="ex", name="ex")
                        nc.scalar.activation(ex[:], sc[:], Exp, scale=0.125)
                        if pend is not None:
                            pm, pex = pend
                            for e in range(2):
                                nc.tensor.matmul(
                                    av[e][:],
                                    vt[pm][:, 65 * (2 * p + e):65 * (2 * p + e) + 65],
                                    pex[:, e * 512:(e + 1) * 512],
                                    start=(pm == 0), stop=(pm == n_m - 1))
                        pend = (m, ex)
                    pm, pex = pend
                    for e in range(2):
                        nc.tensor.matmul(
                            av[e][:],
                            vt[pm][:, 65 * (2 * p + e):65 * (2 * p + e) + 65],
                            pex[:, e * 512:(e + 1) * 512],
                            start=(pm == 0), stop=(pm == n_m - 1))
                    # stage av in SBUF; route its denominator row (base
                    # partition 64, which matmul rhs allows) into partition
                    # 2p+e of the shared den psum tile via a one-hot K=1 MM
                    for e in range(2):
                        r = 2 * p + e
                        asb = avsb.tile([65, 512], f16, tag="asb", name="asb")
                        nc.vector.tensor_copy(asb[:], av[e][:])
                        nc.tensor.matmul(den[:], oneh8_t[64:65, 8 * r:8 * r + 8],
                                         asb[64:65, :],
                                         start=(r == 0), stop=(r == 7))
                        asb_all[(p, e)] = asb
                # one batched reciprocal for all 8 heads of this j chunk
                rinv = rcp.tile([8, 512], f16, tag="rinv", name="rinv")
                with nc.allow_low_precision(
                        reason="softmax denominators are O(1..3e4); fp16 "
                               "reciprocal keeps ~5e-4 rel err"):
                    nc.vector.reciprocal(rinv[:], den[:])
                # normalize: ctx^T[dk, sq] = av[0:64] * (1/av[64]) per head
                for p in range(NPAIR):
                    for e in range(2):
                        r = 2 * p + e
                        bc = avps.tile([65, 512], f32, tag="av", name="bc")
                        nc.tensor.matmul(bc[0:64, :],
                                         selh_t[:, 64 * r:64 * r + 64],
                                         rinv[:], start=True, stop=True)
                        nc.vector.tensor_mul(
                            ctxt[p][64 * e:64 * e + 64, j * 512:(j + 1) * 512],
                            asb_all[(p, e)][0:64, :], bc[0:64, :])

                # output projection for the sq rows finished in this j chunk
                for si in range(4 * j, 4 * j + 4):
                    ot = osb.tile([P, D], f32, tag="ot", name="ot")
                    for h in range(D // 512):
                        ps = ps3.tile([P, 512], f32, tag="av", name="ps3")
                        for p in range(NPAIR):
                            nc.tensor.matmul(
                                ps[:],
                                ctxt[p][:, si * P:(si + 1) * P],
                                wot[p][h][:],
                                start=(p == 0), stop=(p == NPAIR - 1))
                        nc.vector.tensor_copy(ot[:, h * 512:(h + 1) * 512], ps[:])
                    nc.sync.dma_start(out.ap()[si * P:(si + 1) * P, :], ot[:])

    nc.compile()
    return nc


def _get_nc(s_len, causal):
    key = (s_len, causal)
    if key not in _BUILD_CACHE:
        _BUILD_CACHE[key] = _build(s_len, causal)
    return _BUILD_CACHE[key]


def kernel(query, key, value, mask, Wq, bq, Wk, bk, Wv, bv, Wo, bo):
    import ml_dtypes
    from concourse.bass_utils import run_bass_kernel_spmd

    query = np.asarray(query, dtype=np.float32)
    key = np.asarray(key, dtype=np.float32)
    value = np.asarray(value, dtype=np.float32)
    mask = np.asarray(mask, dtype=np.float32)
    Wq, Wk, Wv, Wo = (np.asarray(w, dtype=np.float32) for w in (Wq, Wk, Wv, Wo))
    bq, bk, bv, bo = (np.asarray(b, dtype=np.float32) for b in (bq, bk, bv, bo))

    b_sz, s_len, d = query.shape
    m2 = mask.reshape(s_len, s_len)
    causal = bool(
        np.array_equal(m2, np.triu(np.ones((s_len, s_len), np.float32), k=1)))

    nc = _get_nc(s_len, causal)

    f16 = np.float16
    ones_row = np.ones((1, 512), f16)
    ones_col = np.ones((1, P), f16)
    ones_vcol = np.ones((P, 8), f16)
    selh = np.zeros((8, 8 * 64), f16)
    for r in range(8):
        selh[r, 64 * r:64 * r + 64] = 1.0
    oneh8 = np.zeros((1, 64), f16)
    oneh8[0, 9 * np.arange(8)] = 1.0
    if causal:
        # maskd[d][p, c] = MASK_NEG where p + 128*d > c
        pp = np.arange(P)[:, None]
        cc = np.arange(512)[None, :]
        maskd = np.concatenate(
            [np.where(pp + P * dd > cc, MASK_NEG, 0.0) for dd in range(4)],
            axis=0).astype(ml_dtypes.bfloat16)
    else:
        maskt = (m2.T * MASK_NEG).astype(ml_dtypes.bfloat16)

    in_maps = []
    for c in range(N_CORES):
        b = c // 2
        g = c % 2
        cols = slice(GD * g, GD * g + GD)
        im = {
            "xq": np.ascontiguousarray(query[b].T).astype(f16),
            "xk": np.ascontiguousarray(key[b].T).astype(f16),
            "xv": np.ascontiguousarray(value[b].T).astype(f16),
            "wq": np.ascontiguousarray(Wq[:, cols]).astype(f16),
            "wk": np.ascontiguousarray(Wk[:, cols]).astype(f16),
            "wv": np.ascontiguousarray(Wv[:, cols]).astype(f16),
            "wo": np.ascontiguousarray(Wo[cols, :]).astype(f16),
            "bq": bq[cols].reshape(1, GD).astype(f16),
            "bk": bk[cols].reshape(1, GD).astype(f16),
            "bv": bv[cols].reshape(1, GD).astype(f16),
            "ones_row": ones_row,
            "ones_col": ones_col,
            "ones_vcol": ones_vcol,
            "selh": selh,
            "oneh8": oneh8,
        }
        if causal:
            im["maskd"] = maskd
        else:
            im["maskt"] = maskt
        in_maps.append(im)

    res = run_bass_kernel_spmd(nc, in_maps, list(range(N_CORES)))

    out = np.empty((b_sz, s_len, d), np.float32)
    for b in range(b_sz):
        out[b] = res.results[2 * b]["out"] + res.results[2 * b + 1]["out"] + bo
    return out


# revision 27
# speedup vs baseline: 1.6630x; 1.1812x over previous
"""Multi-head causal attention (B=4, S=2048, D=1024, H=16) on 8 Trainium2
NeuronCores via Bass/Tile.

Sharding: core c handles batch b = c//2 and head-group g = c%2 (8 heads,
i.e. columns [512g, 512g+512) of Wq/Wk/Wv and rows [512g, 512g+512) of Wo).
Each core computes its 8 heads' attention and a partial output projection
[S, D]; the host sums the two head-group partials per batch and adds bo.

Matmul operands are fp16 (full-rate 1 cycle/row on the PE; fp32 accumulate
in PSUM); softmax runs in fp32 on ACT/DVE. All values are O(100) or less so
fp16 range is safe, and fp16's 10-bit mantissa keeps the end-to-end error
around 5e-4. Layouts keep every matmul at N=512 moving columns:
  qT/kT:  [dk, s]  (projection emitted transposed: lhsT=W chunk, rhs=X^T)
  v:      [s, dk]  interleaved with a ones column per head ([..v_h.., 1])
          so the attention-V matmul also produces the softmax row-sums
  scores: [sk, sq] (transposed; lhsT=kT chunk, rhs=qT) -> exp -> expT
  AV:     av[65, sq] += v_aug^T @ expT  (row 64 = softmax denominators)
  out:    partial[sq, :] = ctx^T.T @ Wo  (ctx^T is exactly the AV output)
"""

import os
import sys
import numpy as np

for _p in ("/opt/trn_rl_repo", "/root/.axon_site/_ro/trn_rl_repo"):
    if _p not in sys.path:
        sys.path.append(_p)

B, S_FULL, D, H, DK = 4, 2048, 1024, 16, 64
GD = 512          # dk span per core (8 heads)
P = 128
NPAIR = GD // P   # 4 head-pairs per core
N_CORES = 8
MASK_NEG = -8.0e9  # multiplied by the 0.125 softmax scale inside exp -> -1e9

_BUILD_CACHE = {}


def _build(s_len, causal):
    from contextlib import ExitStack

    import concourse.tile as tile
    from concourse import bacc, mybir

    dt = mybir.dt
    f32, f16, bf16 = dt.float32, dt.float16, dt.bfloat16
    Exp = mybir.ActivationFunctionType.Exp

    S = s_len
    SJ = S // 512     # 512-wide sq chunks
    SM = S // P       # 128-wide sk chunks
    DC = D // P       # contraction chunks for the projections

    nc = bacc.Bacc("TRN2", target_bir_lowering=False, debug=False,
                   num_devices=N_CORES)

    xq = nc.dram_tensor("xq", [D, S], f16, kind="ExternalInput")
    xk = nc.dram_tensor("xk", [D, S], f16, kind="ExternalInput")
    xv = nc.dram_tensor("xv", [D, S], f16, kind="ExternalInput")
    wq = nc.dram_tensor("wq", [D, GD], f16, kind="ExternalInput")
    wk = nc.dram_tensor("wk", [D, GD], f16, kind="ExternalInput")
    wv = nc.dram_tensor("wv", [D, GD], f16, kind="ExternalInput")
    wo = nc.dram_tensor("wo", [GD, D], f16, kind="ExternalInput")
    bq = nc.dram_tensor("bq", [1, GD], f16, kind="ExternalInput")
    bk = nc.dram_tensor("bk", [1, GD], f16, kind="ExternalInput")
    bv = nc.dram_tensor("bv", [1, GD], f16, kind="ExternalInput")
    ones_row = nc.dram_tensor("ones_row", [1, 512], f16, kind="ExternalInput")
    ones_col = nc.dram_tensor("ones_col", [1, P], f16, kind="ExternalInput")
    ones_vcol = nc.dram_tensor("ones_vcol", [P, 8], f16, kind="ExternalInput")
    # selh[:, 64r:64r+64] is row-r-one-hot: selects head r's reciprocal row
    # and broadcasts it over 64 partitions in one K=8 matmul
    selh = nc.dram_tensor("selh", [8, 8 * 64], f16, kind="ExternalInput")
    # oneh8[0, 8r:8r+8] is the one-hot row e_r: routes head r's softmax
    # denominator row into partition r of the gathered [8, 512] psum tile
    oneh8 = nc.dram_tensor("oneh8", [1, 64], f16, kind="ExternalInput")
    if causal:
        # 4 canonical diagonal-band blocks: block d, entry [p, c] masked
        # when p + 128*d > c  (value MASK_NEG, else 0)
        maskd = nc.dram_tensor("maskd", [4 * P, 512], bf16, kind="ExternalInput")
    else:
        # full transposed mask [sk, sq] * MASK_NEG
        maskt = nc.dram_tensor("maskt", [S, S], bf16, kind="ExternalInput")
    out = nc.dram_tensor("out", [S, D], f32, kind="ExternalOutput")

    with tile.TileContext(nc) as tc, ExitStack() as ctx0:
        persist = ctx0.enter_context(tc.tile_pool(name="persist", bufs=1))

        qTt = [persist.tile([P, S], f16, tag=f"qT{p}", name=f"qT{p}")
               for p in range(NPAIR)]
        kTt = [persist.tile([P, S], f16, tag=f"kT{p}", name=f"kT{p}")
               for p in range(NPAIR)]
        vt = [persist.tile([P, 8 * 65], f16, tag=f"v{m}", name=f"v{m}")
              for m in range(SM)]
        ones_row_t = persist.tile([1, 512], f16, tag="ones_row")
        ones_col_t = persist.tile([1, P], f16, tag="ones_col")
        bq_t = persist.tile([1, GD], f16, tag="bq")
        bk_t = persist.tile([1, GD], f16, tag="bk")
        bv_t = persist.tile([1, GD], f16, tag="bv")
        selh_t = persist.tile([8, 8 * 64], f16, tag="selh")
        # staged at partition 64 so its base matches asb[64:65] in the
        # denominator-gather matmul (lhsT/rhs bases must agree)
        oneh8_t = persist.tile([65, 64], f16, tag="oneh8")
        nc.sync.dma_start(selh_t[:], selh.ap())
        nc.sync.dma_start(oneh8_t[64:65, :], oneh8.ap())
        nc.sync.dma_start(ones_row_t[:], ones_row.ap())
        nc.sync.dma_start(ones_col_t[:], ones_col.ap())
        nc.sync.dma_start(bq_t[:], bq.ap())
        nc.sync.dma_start(bk_t[:], bk.ap())
        nc.sync.dma_start(bv_t[:], bv.ap())
        if causal:
            # one [128,128] block covers every diagonal mixed window:
            # within the window the pattern is always "masked iff p > c"
            mask128_t = persist.tile([P, P], bf16, tag="mask128")
            nc.sync.dma_start(mask128_t[:], maskd.ap()[0:P, 0:P])

        # ---------------- phase 1: projections ----------------
        # full-row X^T staging ([128, S] fp16); X loads dispatch from the
        # (otherwise idle) scalar engine's DGE so SP isn't a serial choke
        with ExitStack() as ctx1:
            xpool = ctx1.enter_context(tc.tile_pool(name="xt", bufs=16))
            wpool = ctx1.enter_context(tc.tile_pool(name="wt", bufs=24))
            ps1 = ctx1.enter_context(tc.tile_pool(name="ps1", bufs=3, space="PSUM"))

            # q and k projections, emitted transposed [dk, s]
            for x_d, w_d, b_t, outT in ((xq, wq, bq_t, qTt), (xk, wk, bk_t, kTt)):
                wt = []
                xt = []
                for dc in range(DC):
                    w = wpool.tile([P, GD], f16, tag="wt", name="wt")
                    nc.sync.dma_start(w[:], w_d.ap()[dc * P:(dc + 1) * P, :])
                    wt.append(w)
                    t = xpool.tile([P, S], f16, tag="xt", name="xt")
                    nc.scalar.dma_start(t[:], x_d.ap()[dc * P:(dc + 1) * P, :])
                    xt.append(t)
                for sj in range(SJ):
                    for i in range(NPAIR):
                        ps = ps1.tile([P, 512], f32, tag="ps1", name="ps1")
                        for dc in range(DC):
                            nc.tensor.matmul(
                                ps[:],
                                wt[dc][:, i * P:(i + 1) * P],
                                xt[dc][:, sj * 512:(sj + 1) * 512],
                                start=(dc == 0), stop=False)
                        nc.tensor.matmul(
                            ps[:], b_t[0:1, i * P:(i + 1) * P], ones_row_t[:],
                            start=False, stop=True)
                        nc.vector.tensor_copy(
                            outT[i][:, sj * 512:(sj + 1) * 512], ps[:])

            # v projection, natural layout [s, dk] with interleaved ones col
            wt = []
            xt = []
            for dc in range(DC):
                w = wpool.tile([P, GD], f16, tag="wt", name="wt")
                nc.sync.dma_start(w[:], wv.ap()[dc * P:(dc + 1) * P, :])
                wt.append(w)
                t = xpool.tile([P, S], f16, tag="xt", name="xt")
                nc.scalar.dma_start(t[:], xv.ap()[dc * P:(dc + 1) * P, :])
                xt.append(t)
            for si in range(SM):
                ps = ps1.tile([P, 512], f32, tag="ps1", name="ps1")
                for dc in range(DC):
                    nc.tensor.matmul(
                        ps[:],
                        xt[dc][:, si * P:(si + 1) * P],
                        wt[dc][:],
                        start=(dc == 0), stop=False)
                nc.tensor.matmul(ps[:], ones_col_t[:], bv_t[:],
                                 start=False, stop=True)
                v3 = vt[si][:].rearrange("p (h c) -> p h c", h=8)
                nc.vector.tensor_copy(
                    v3[:, :, 0:64],
                    ps[:].rearrange("p (h c) -> p h c", h=8))
                nc.sync.dma_start(v3[:, :, 64:65], ones_vcol.ap()[:, :, None])

        # ---------------- phases 2+3: attention + output projection ------
        with ExitStack() as ctx2:
            ctxpool = ctx2.enter_context(tc.tile_pool(name="ctxp", bufs=1))
            ctxt = [ctxpool.tile([P, S], f16, tag=f"ctx{p}", name=f"ctx{p}")
                    for p in range(NPAIR)]
            wopool = ctx2.enter_context(tc.tile_pool(name="wo", bufs=1))
            scps = ctx2.enter_context(tc.tile_pool(name="scps", bufs=2, space="PSUM"))
            avps = ctx2.enter_context(tc.tile_pool(name="avps", bufs=4, space="PSUM"))
            ps3 = avps
            expp = ctx2.enter_context(tc.tile_pool(name="expp", bufs=4))
            avsb = ctx2.enter_context(tc.tile_pool(name="avsb", bufs=10))
            rcp = ctx2.enter_context(tc.tile_pool(name="rcp", bufs=2))
            osb = ctx2.enter_context(tc.tile_pool(name="osb", bufs=3))
            if not causal:
                mpool = ctx2.enter_context(tc.tile_pool(name="mp", bufs=SM + 2))

            wot = [[wopool.tile([P, 512], f16, tag=f"wo{p}_{h}", name=f"wo{p}_{h}")
                    for h in range(D // 512)] for p in range(NPAIR)]
            for p in range(NPAIR):
                for h in range(D // 512):
                    nc.sync.dma_start(
                        wot[p][h][:],
                        wo.ap()[p * P:(p + 1) * P, h * 512:(h + 1) * 512])

            def emit_outproj(j):
                # output projection for the sq rows of chunk j
                for si in range(4 * j, 4 * j + 4):
                    ot = osb.tile([P, D], f32, tag="ot", name="ot")
                    for h in range(D // 512):
                        ps = ps3.tile([P, 512], f32, tag="av", name="ps3")
                        for p in range(NPAIR):
                            nc.tensor.matmul(
                                ps[:],
                                ctxt[p][:, si * P:(si + 1) * P],
                                wot[p][h][:],
                                start=(p == 0), stop=(p == NPAIR - 1))
                        nc.vector.tensor_copy(ot[:, h * 512:(h + 1) * 512], ps[:])
                    nc.sync.dma_start(out.ap()[si * P:(si + 1) * P, :], ot[:])

            prev_j = None
            for j in range(SJ):
                n_m = 4 * (j + 1) if causal else SM
                if not causal:
                    mt = []
                    for m in range(SM):
                        t = mpool.tile([P, 512], bf16, tag="mt", name="mt")
                        nc.sync.dma_start(
                            t[:], maskt.ap()[m * P:(m + 1) * P,
                                             j * 512:(j + 1) * 512])
                        mt.append(t)
                den = avps.tile([8, 512], f32, tag="av", name="den")
                asb_all = {}
                for p in range(NPAIR):
                    av = [avps.tile([65, 512], f32, tag="av", name="av")
                          for _ in range(2)]
                    pend = None  # (m, exp_tile, c0) awaiting its AV matmuls
                    for m in range(n_m):
                        # causal diagonal block d: columns [0, 128d) of this
                        # sq chunk are fully masked -> compute only the
                        # suffix [c0, 512) in scores/exp/AV; the mixed
                        # 128-col window gets the shared p>c mask block
                        d = m - 4 * j if causal else -1
                        c0 = 128 * d if d > 0 else 0
                        nv = 512 - c0
                        sc = scps.tile([P, 1024], f32, tag="sc", name="sc")
                        for e in range(2):
                            nc.tensor.matmul(
                                sc[:, e * 512 + c0:(e + 1) * 512],
                                kTt[p][64 * e:64 * e + 64, m * P:(m + 1) * P],
                                qTt[p][64 * e:64 * e + 64,
                                       j * 512 + c0:(j + 1) * 512],
                                start=True, stop=True)
                        sc3 = sc[:].rearrange("p (e c) -> p e c", e=2)
                        if causal:
                            if d >= 0:
                                nc.vector.tensor_add(
                                    sc3[:, :, c0:c0 + P], sc3[:, :, c0:c0 + P],
                                    mask128_t[:][:, None, :].broadcast_to(
                                        [P, 2, P]))
                        else:
                            nc.vector.tensor_add(
                                sc3, sc3,
                                mt[m][:][:, None, :].broadcast_to([P, 2, 512]))
                        ex = expp.tile([P, 1024], f16, tag="ex", name="ex")
                        ex3 = ex[:].rearrange("p (e c) -> p e c", e=2)
                        nc.scalar.activation(ex3[:, :, c0:512],
                                             sc3[:, :, c0:512], Exp, scale=0.125)
                        if pend is not None:
                            pm, pex, pc0 = pend
                            for e in range(2):
                                nc.tensor.matmul(
                                    av[e][:, pc0:512],
                                    vt[pm][:, 65 * (2 * p + e):65 * (2 * p + e) + 65],
                                    pex[:, e * 512 + pc0:(e + 1) * 512],
                                    start=(pm == 0), stop=(pm == n_m - 1))
                        pend = (m, ex, c0)
                    pm, pex, pc0 = pend
                    for e in range(2):
                        nc.tensor.matmul(
                            av[e][:, pc0:512],
                            vt[pm][:, 65 * (2 * p + e):65 * (2 * p + e) + 65],
                            pex[:, e * 512 + pc0:(e + 1) * 512],
                            start=(pm == 0), stop=(pm == n_m - 1))
                    # stage av in SBUF; route its denominator row (base
                    # partition 64, which matmul rhs allows) into partition
                    # 2p+e of the shared den psum tile via a one-hot K=1 MM
                    for e in range(2):
                        r = 2 * p + e
                        asb = avsb.tile([65, 512], f16, tag="asb", name="asb")
                        nc.vector.tensor_copy(asb[:], av[e][:])
                        nc.tensor.matmul(den[:], oneh8_t[64:65, 8 * r:8 * r + 8],
                                         asb[64:65, :],
                                         start=(r == 0), stop=(r == 7))
                        asb_all[(p, e)] = asb
                # out-projection of the previous chunk lands here: it gives
                # the PE independent work while this chunk's reciprocal and
                # normalization chain runs on DVE
                if prev_j is not None:
                    emit_outproj(prev_j)
                # one batched reciprocal for all 8 heads of this j chunk
                rinv = rcp.tile([8, 512], f16, tag="rinv", name="rinv")
                with nc.allow_low_precision(
                        reason="softmax denominators are O(1..3e4); fp16 "
                               "reciprocal keeps ~5e-4 rel err"):
                    nc.vector.reciprocal(rinv[:], den[:])
                # normalize: ctx^T[dk, sq] = av[0:64] * (1/av[64]) per head
                for p in range(NPAIR):
                    for e in range(2):
                        r = 2 * p + e
                        bc = avps.tile([65, 512], f32, tag="av", name="bc")
                        nc.tensor.matmul(bc[0:64, :],
                                         selh_t[:, 64 * r:64 * r + 64],
                                         rinv[:], start=True, stop=True)
                        nc.vector.tensor_mul(
                            ctxt[p][64 * e:64 * e + 64, j * 512:(j + 1) * 512],
                            asb_all[(p, e)][0:64, :], bc[0:64, :])
                prev_j = j
            emit_outproj(prev_j)

    nc.compile()
    return nc


def _get_nc(s_len, causal):
    key = (s_len, causal)
    if key not in _BUILD_CACHE:
        _BUILD_CACHE[key] = _build(s_len, causal)
    return _BUILD_CACHE[key]


def kernel(query, key, value, mask, Wq, bq, Wk, bk, Wv, bv, Wo, bo):
    import ml_dtypes
    from concourse.bass_utils import run_bass_kernel_spmd

    query = np.asarray(query, dtype=np.float32)
    key = np.asarray(key, dtype=np.float32)
    value = np.asarray(value, dtype=np.float32)
    mask = np.asarray(mask, dtype=np.float32)
    Wq, Wk, Wv, Wo = (np.asarray(w, dtype=np.float32) for w in (Wq, Wk, Wv, Wo))
    bq, bk, bv, bo = (np.asarray(b, dtype=np.float32) for b in (bq, bk, bv, bo))

    b_sz, s_len, d = query.shape
    m2 = mask.reshape(s_len, s_len)
    causal = bool(
        np.array_equal(m2, np.triu(np.ones((s_len, s_len), np.float32), k=1)))

    nc = _get_nc(s_len, causal)

    f16 = np.float16
    ones_row = np.ones((1, 512), f16)
    ones_col = np.ones((1, P), f16)
    ones_vcol = np.ones((P, 8), f16)
    selh = np.zeros((8, 8 * 64), f16)
    for r in range(8):
        selh[r, 64 * r:64 * r + 64] = 1.0
    oneh8 = np.zeros((1, 64), f16)
    oneh8[0, 9 * np.arange(8)] = 1.0
    if causal:
        # maskd[d][p, c] = MASK_NEG where p + 128*d > c
        pp = np.arange(P)[:, None]
        cc = np.arange(512)[None, :]
        maskd = np.concatenate(
            [np.where(pp + P * dd > cc, MASK_NEG, 0.0) for dd in range(4)],
            axis=0).astype(ml_dtypes.bfloat16)
    else:
        maskt = (m2.T * MASK_NEG).astype(ml_dtypes.bfloat16)

    in_maps = []
    for c in range(N_CORES):
        b = c // 2
        g = c % 2
        cols = slice(GD * g, GD * g + GD)
        im = {
            "xq": np.ascontiguousarray(query[b].T).astype(f16),
            "xk": np.ascontiguousarray(key[b].T).astype(f16),
            "xv": np.ascontiguousarray(value[b].T).astype(f16),
            "wq": np.ascontiguousarray(Wq[:, cols]).astype(f16),
            "wk": np.ascontiguousarray(Wk[:, cols]).astype(f16),
            "wv": np.ascontiguousarray(Wv[:, cols]).astype(f16),
            "wo": np.ascontiguousarray(Wo[cols, :]).astype(f16),
            "bq": bq[cols].reshape(1, GD).astype(f16),
            "bk": bk[cols].reshape(1, GD).astype(f16),
            "bv": bv[cols].reshape(1, GD).astype(f16),
            "ones_row": ones_row,
            "ones_col": ones_col,
            "ones_vcol": ones_vcol,
            "selh": selh,
            "oneh8": oneh8,
        }
        if causal:
            im["maskd"] = maskd
        else:
            im["maskt"] = maskt
        in_maps.append(im)

    res = run_bass_kernel_spmd(nc, in_maps, list(range(N_CORES)))

    out = np.empty((b_sz, s_len, d), np.float32)
    for b in range(b_sz):
        out[b] = res.results[2 * b]["out"] + res.results[2 * b + 1]["out"] + bo
    return out


# revision 30
# speedup vs baseline: 1.7681x; 1.0632x over previous
"""Multi-head causal attention (B=4, S=2048, D=1024, H=16) on 8 Trainium2
NeuronCores via Bass/Tile.

Sharding: core c handles batch b = c//2 and head-group g = c%2 (8 heads,
i.e. columns [512g, 512g+512) of Wq/Wk/Wv and rows [512g, 512g+512) of Wo).
Each core computes its 8 heads' attention and a partial output projection
[S, D]; the host sums the two head-group partials per batch and adds bo.

Matmul operands are fp16 (full-rate 1 cycle/row on the PE; fp32 accumulate
in PSUM); softmax runs in fp32 on ACT/DVE. All values are O(100) or less so
fp16 range is safe, and fp16's 10-bit mantissa keeps the end-to-end error
around 5e-4. Layouts keep every matmul at N=512 moving columns:
  qT/kT:  [dk, s]  (projection emitted transposed: lhsT=W chunk, rhs=X^T)
  v:      [s, dk]  interleaved with a ones column per head ([..v_h.., 1])
          so the attention-V matmul also produces the softmax row-sums
  scores: [sk, sq] (transposed; lhsT=kT chunk, rhs=qT) -> exp -> expT
  AV:     av[65, sq] += v_aug^T @ expT  (row 64 = softmax denominators)
  out:    partial[sq, :] = ctx^T.T @ Wo  (ctx^T is exactly the AV output)
"""

import os
import sys
import numpy as np

for _p in ("/opt/trn_rl_repo", "/root/.axon_site/_ro/trn_rl_repo"):
    if _p not in sys.path:
        sys.path.append(_p)

B, S_FULL, D, H, DK = 4, 2048, 1024, 16, 64
GD = 512          # dk span per core (8 heads)
P = 128
NPAIR = GD // P   # 4 head-pairs per core
N_CORES = 8
MASK_NEG = -8.0e9  # multiplied by the 0.125 softmax scale inside exp -> -1e9

_BUILD_CACHE = {}


def _build(s_len, causal):
    from contextlib import ExitStack

    import concourse.tile as tile
    from concourse import bacc, mybir

    dt = mybir.dt
    f32, f16, bf16 = dt.float32, dt.float16, dt.bfloat16
    Exp = mybir.ActivationFunctionType.Exp

    S = s_len
    SJ = S // 512     # 512-wide sq chunks
    SM = S // P       # 128-wide sk chunks
    DC = D // P       # contraction chunks for the projections

    nc = bacc.Bacc("TRN2", target_bir_lowering=False, debug=False,
                   num_devices=N_CORES)

    xq = nc.dram_tensor("xq", [D, S], f16, kind="ExternalInput")
    xk = nc.dram_tensor("xk", [D, S], f16, kind="ExternalInput")
    xv = nc.dram_tensor("xv", [D, S], f16, kind="ExternalInput")
    wq = nc.dram_tensor("wq", [D, GD], f16, kind="ExternalInput")
    wk = nc.dram_tensor("wk", [D, GD], f16, kind="ExternalInput")
    wv = nc.dram_tensor("wv", [D, GD], f16, kind="ExternalInput")
    wo = nc.dram_tensor("wo", [GD, D], f16, kind="ExternalInput")
    bq = nc.dram_tensor("bq", [1, GD], f16, kind="ExternalInput")
    bk = nc.dram_tensor("bk", [1, GD], f16, kind="ExternalInput")
    bv = nc.dram_tensor("bv", [1, GD], f16, kind="ExternalInput")
    ones_row = nc.dram_tensor("ones_row", [1, 512], f16, kind="ExternalInput")
    ones_col = nc.dram_tensor("ones_col", [1, P], f16, kind="ExternalInput")
    ones_vcol = nc.dram_tensor("ones_vcol", [P, 8], f16, kind="ExternalInput")
    # selh[:, 64r:64r+64] is row-r-one-hot: selects head r's reciprocal row
    # and broadcasts it over 64 partitions in one K=8 matmul
    selh = nc.dram_tensor("selh", [8, 8 * 64], f16, kind="ExternalInput")
    # oneh8[0, 8r:8r+8] is the one-hot row e_r: routes head r's softmax
    # denominator row into partition r of the gathered [8, 512] psum tile
    oneh8 = nc.dram_tensor("oneh8", [1, 64], f16, kind="ExternalInput")
    if causal:
        # 4 canonical diagonal-band blocks: block d, entry [p, c] masked
        # when p + 128*d > c  (value MASK_NEG, else 0)
        maskd = nc.dram_tensor("maskd", [4 * P, 512], bf16, kind="ExternalInput")
    else:
        # full transposed mask [sk, sq] * MASK_NEG
        maskt = nc.dram_tensor("maskt", [S, S], bf16, kind="ExternalInput")
    out = nc.dram_tensor("out", [S, D], f32, kind="ExternalOutput")

    with tile.TileContext(nc) as tc, ExitStack() as ctx0:
        persist = ctx0.enter_context(tc.tile_pool(name="persist", bufs=1))

        qTt = [persist.tile([P, S], f16, tag=f"qT{p}", name=f"qT{p}")
               for p in range(NPAIR)]
        kTt = [persist.tile([P, S], f16, tag=f"kT{p}", name=f"kT{p}")
               for p in range(NPAIR)]
        vt = [persist.tile([P, 8 * 65], f16, tag=f"v{m}", name=f"v{m}")
              for m in range(SM)]
        ones_row_t = persist.tile([1, 512], f16, tag="ones_row")
        ones_col_t = persist.tile([1, P], f16, tag="ones_col")
        bq_t = persist.tile([1, GD], f16, tag="bq")
        bk_t = persist.tile([1, GD], f16, tag="bk")
        bv_t = persist.tile([1, GD], f16, tag="bv")
        selh_t = persist.tile([8, 8 * 64], f16, tag="selh")
        # staged at partition 64 so its base matches asb[64:65] in the
        # denominator-gather matmul (lhsT/rhs bases must agree)
        oneh8_t = persist.tile([65, 64], f16, tag="oneh8")
        nc.sync.dma_start(selh_t[:], selh.ap())
        nc.sync.dma_start(oneh8_t[64:65, :], oneh8.ap())
        nc.sync.dma_start(ones_row_t[:], ones_row.ap())
        nc.sync.dma_start(ones_col_t[:], ones_col.ap())
        nc.sync.dma_start(bq_t[:], bq.ap())
        nc.sync.dma_start(bk_t[:], bk.ap())
        nc.sync.dma_start(bv_t[:], bv.ap())
        if causal:
            # one [128,128] block covers every diagonal mixed window:
            # within the window the pattern is always "masked iff p > c"
            mask128_t = persist.tile([P, P], bf16, tag="mask128")
            nc.sync.dma_start(mask128_t[:], maskd.ap()[0:P, 0:P])

        # ---------------- phase 1: projections ----------------
        # full-row X^T staging ([128, S] fp16); X loads dispatch from the
        # (otherwise idle) scalar engine's DGE so SP isn't a serial choke
        with ExitStack() as ctx1:
            xpool = ctx1.enter_context(tc.tile_pool(name="xt", bufs=16))
            wpool = ctx1.enter_context(tc.tile_pool(name="wt", bufs=24))
            ps1 = ctx1.enter_context(tc.tile_pool(name="ps1", bufs=3, space="PSUM"))

            # q and k projections, emitted transposed [dk, s]
            for x_d, w_d, b_t, outT in ((xq, wq, bq_t, qTt), (xk, wk, bk_t, kTt)):
                wt = []
                xt = []
                for dc in range(DC):
                    w = wpool.tile([P, GD], f16, tag="wt", name="wt")
                    nc.sync.dma_start(w[:], w_d.ap()[dc * P:(dc + 1) * P, :])
                    wt.append(w)
                    t = xpool.tile([P, S], f16, tag="xt", name="xt")
                    eng = nc.scalar if dc % 2 == 0 else nc.gpsimd
                    eng.dma_start(t[:], x_d.ap()[dc * P:(dc + 1) * P, :])
                    xt.append(t)
                for i in range(NPAIR):
                    for sj in range(SJ):
                        ps = ps1.tile([P, 512], f32, tag="ps1", name="ps1")
                        for dc in range(DC):
                            nc.tensor.matmul(
                                ps[:],
                                wt[dc][:, i * P:(i + 1) * P],
                                xt[dc][:, sj * 512:(sj + 1) * 512],
                                start=(dc == 0), stop=False)
                        nc.tensor.matmul(
                            ps[:], b_t[0:1, i * P:(i + 1) * P], ones_row_t[:],
                            start=False, stop=True)
                        nc.vector.tensor_copy(
                            outT[i][:, sj * 512:(sj + 1) * 512], ps[:])

            # v projection, natural layout [s, dk] with interleaved ones col
            wt = []
            xt = []
            for dc in range(DC):
                w = wpool.tile([P, GD], f16, tag="wt", name="wt")
                nc.sync.dma_start(w[:], wv.ap()[dc * P:(dc + 1) * P, :])
                wt.append(w)
                t = xpool.tile([P, S], f16, tag="xt", name="xt")
                eng = nc.scalar if dc % 2 == 0 else nc.gpsimd
                eng.dma_start(t[:], xv.ap()[dc * P:(dc + 1) * P, :])
                xt.append(t)
            for si in range(SM):
                ps = ps1.tile([P, 512], f32, tag="ps1", name="ps1")
                for dc in range(DC):
                    nc.tensor.matmul(
                        ps[:],
                        xt[dc][:, si * P:(si + 1) * P],
                        wt[dc][:],
                        start=(dc == 0), stop=False)
                nc.tensor.matmul(ps[:], ones_col_t[:], bv_t[:],
                                 start=False, stop=True)
                v3 = vt[si][:].rearrange("p (h c) -> p h c", h=8)
                nc.vector.tensor_copy(
                    v3[:, :, 0:64],
                    ps[:].rearrange("p (h c) -> p h c", h=8))
                nc.sync.dma_start(v3[:, :, 64:65], ones_vcol.ap()[:, :, None])

        # ---------------- phases 2+3: attention + output projection ------
        with ExitStack() as ctx2:
            ctxpool = ctx2.enter_context(tc.tile_pool(name="ctxp", bufs=1))
            ctxt = [ctxpool.tile([P, S], f16, tag=f"ctx{p}", name=f"ctx{p}")
                    for p in range(NPAIR)]
            wopool = ctx2.enter_context(tc.tile_pool(name="wo", bufs=1))
            scps = ctx2.enter_context(tc.tile_pool(name="scps", bufs=2, space="PSUM"))
            avps = ctx2.enter_context(tc.tile_pool(name="avps", bufs=4, space="PSUM"))
            ps3 = avps
            expp = ctx2.enter_context(tc.tile_pool(name="expp", bufs=4))
            avsb = ctx2.enter_context(tc.tile_pool(name="avsb", bufs=10))
            rcp = ctx2.enter_context(tc.tile_pool(name="rcp", bufs=2))
            osb = ctx2.enter_context(tc.tile_pool(name="osb", bufs=3))
            if not causal:
                mpool = ctx2.enter_context(tc.tile_pool(name="mp", bufs=SM + 2))

            wot = [[wopool.tile([P, 512], f16, tag=f"wo{p}_{h}", name=f"wo{p}_{h}")
                    for h in range(D // 512)] for p in range(NPAIR)]
            for p in range(NPAIR):
                for h in range(D // 512):
                    nc.sync.dma_start(
                        wot[p][h][:],
                        wo.ap()[p * P:(p + 1) * P, h * 512:(h + 1) * 512])

            def emit_outproj_si(si):
                # output projection for one 128-row block of sq
                if True:
                    ot = osb.tile([P, D], f32, tag="ot", name="ot")
                    for h in range(D // 512):
                        ps = ps3.tile([P, 512], f32, tag="av", name="ps3")
                        for p in range(NPAIR):
                            nc.tensor.matmul(
                                ps[:],
                                ctxt[p][:, si * P:(si + 1) * P],
                                wot[p][h][:],
                                start=(p == 0), stop=(p == NPAIR - 1))
                        nc.vector.tensor_copy(ot[:, h * 512:(h + 1) * 512], ps[:])
                    nc.sync.dma_start(out.ap()[si * P:(si + 1) * P, :], ot[:])

            pending_si = []
            for j in range(SJ):
                n_m = 4 * (j + 1) if causal else SM
                if not causal:
                    mt = []
                    for m in range(SM):
                        t = mpool.tile([P, 512], bf16, tag="mt", name="mt")
                        nc.sync.dma_start(
                            t[:], maskt.ap()[m * P:(m + 1) * P,
                                             j * 512:(j + 1) * 512])
                        mt.append(t)
                den = avps.tile([8, 512], f32, tag="av", name="den")
                asb_all = {}
                for p in range(NPAIR):
                    # out-projection rows of the previous chunk interleave
                    # here: independent PE work that covers this chunk's
                    # exp/normalization latency
                    if pending_si:
                        emit_outproj_si(pending_si.pop(0))
                    av = [avps.tile([65, 512], f32, tag="av", name="av")
                          for _ in range(2)]
                    pend = []  # (m, exp_tile, c0) awaiting their AV matmuls
                    for m in range(n_m):
                        # causal diagonal block d: columns [0, 128d) of this
                        # sq chunk are fully masked -> compute only the
                        # suffix [c0, 512) in scores/exp/AV; the mixed
                        # 128-col window gets the shared p>c mask block
                        d = m - 4 * j if causal else -1
                        c0 = 128 * d if d > 0 else 0
                        nv = 512 - c0
                        sc = scps.tile([P, 1024], f32, tag="sc", name="sc")
                        for e in range(2):
                            nc.tensor.matmul(
                                sc[:, e * 512 + c0:(e + 1) * 512],
                                kTt[p][64 * e:64 * e + 64, m * P:(m + 1) * P],
                                qTt[p][64 * e:64 * e + 64,
                                       j * 512 + c0:(j + 1) * 512],
                                start=True, stop=True)
                        sc3 = sc[:].rearrange("p (e c) -> p e c", e=2)
                        if causal:
                            if d >= 0:
                                nc.vector.tensor_add(
                                    sc3[:, :, c0:c0 + P], sc3[:, :, c0:c0 + P],
                                    mask128_t[:][:, None, :].broadcast_to(
                                        [P, 2, P]))
                        else:
                            nc.vector.tensor_add(
                                sc3, sc3,
                                mt[m][:][:, None, :].broadcast_to([P, 2, 512]))
                        ex = expp.tile([P, 1024], f16, tag="ex", name="ex")
                        ex3 = ex[:].rearrange("p (e c) -> p e c", e=2)
                        nc.scalar.activation(ex3[:, :, c0:512],
                                             sc3[:, :, c0:512], Exp, scale=0.125)
                        pend.append((m, ex, c0))
                        if len(pend) > 2:
                            pm, pex, pc0 = pend.pop(0)
                            for e in range(2):
                                nc.tensor.matmul(
                                    av[e][:, pc0:512],
                                    vt[pm][:, 65 * (2 * p + e):65 * (2 * p + e) + 65],
                                    pex[:, e * 512 + pc0:(e + 1) * 512],
                                    start=(pm == 0), stop=(pm == n_m - 1))
                    for pm, pex, pc0 in pend:
                        for e in range(2):
                            nc.tensor.matmul(
                                av[e][:, pc0:512],
                                vt[pm][:, 65 * (2 * p + e):65 * (2 * p + e) + 65],
                                pex[:, e * 512 + pc0:(e + 1) * 512],
                                start=(pm == 0), stop=(pm == n_m - 1))
                    # stage av in SBUF; route its denominator row (base
                    # partition 64, which matmul rhs allows) into partition
                    # 2p+e of the shared den psum tile via a one-hot K=1 MM
                    for e in range(2):
                        r = 2 * p + e
                        asb = avsb.tile([65, 512], f16, tag="asb", name="asb")
                        nc.vector.tensor_copy(asb[:], av[e][:])
                        nc.tensor.matmul(den[:], oneh8_t[64:65, 8 * r:8 * r + 8],
                                         asb[64:65, :],
                                         start=(r == 0), stop=(r == 7))
                        asb_all[(p, e)] = asb

                # one batched reciprocal for all 8 heads of this j chunk
                rinv = rcp.tile([8, 512], f16, tag="rinv", name="rinv")
                with nc.allow_low_precision(
                        reason="softmax denominators are O(1..3e4); fp16 "
                               "reciprocal keeps ~5e-4 rel err"):
                    nc.vector.reciprocal(rinv[:], den[:])
                # normalize: ctx^T[dk, sq] = av[0:64] * (1/av[64]) per head
                for p in range(NPAIR):
                    for e in range(2):
                        r = 2 * p + e
                        bc = avps.tile([65, 512], f32, tag="av", name="bc")
                        nc.tensor.matmul(bc[0:64, :],
                                         selh_t[:, 64 * r:64 * r + 64],
                                         rinv[:], start=True, stop=True)
                        nc.vector.tensor_mul(
                            ctxt[p][64 * e:64 * e + 64, j * 512:(j + 1) * 512],
                            asb_all[(p, e)][0:64, :], bc[0:64, :])
                pending_si.extend(range(4 * j, 4 * j + 4))
            for si in pending_si:
                emit_outproj_si(si)

    nc.compile()
    return nc


def _get_nc(s_len, causal):
    key = (s_len, causal)
    if key not in _BUILD_CACHE:
        _BUILD_CACHE[key] = _build(s_len, causal)
    return _BUILD_CACHE[key]


def kernel(query, key, value, mask, Wq, bq, Wk, bk, Wv, bv, Wo, bo):
    import ml_dtypes
    from concourse.bass_utils import run_bass_kernel_spmd

    query = np.asarray(query, dtype=np.float32)
    key = np.asarray(key, dtype=np.float32)
    value = np.asarray(value, dtype=np.float32)
    mask = np.asarray(mask, dtype=np.float32)
    Wq, Wk, Wv, Wo = (np.asarray(w, dtype=np.float32) for w in (Wq, Wk, Wv, Wo))
    bq, bk, bv, bo = (np.asarray(b, dtype=np.float32) for b in (bq, bk, bv, bo))

    b_sz, s_len, d = query.shape
    m2 = mask.reshape(s_len, s_len)
    causal = bool(
        np.array_equal(m2, np.triu(np.ones((s_len, s_len), np.float32), k=1)))

    nc = _get_nc(s_len, causal)

    f16 = np.float16
    ones_row = np.ones((1, 512), f16)
    ones_col = np.ones((1, P), f16)
    ones_vcol = np.ones((P, 8), f16)
    selh = np.zeros((8, 8 * 64), f16)
    for r in range(8):
        selh[r, 64 * r:64 * r + 64] = 1.0
    oneh8 = np.zeros((1, 64), f16)
    oneh8[0, 9 * np.arange(8)] = 1.0
    if causal:
        # maskd[d][p, c] = MASK_NEG where p + 128*d > c
        pp = np.arange(P)[:, None]
        cc = np.arange(512)[None, :]
        maskd = np.concatenate(
            [np.where(pp + P * dd > cc, MASK_NEG, 0.0) for dd in range(4)],
            axis=0).astype(ml_dtypes.bfloat16)
    else:
        maskt = (m2.T * MASK_NEG).astype(ml_dtypes.bfloat16)

    in_maps = []
    for c in range(N_CORES):
        b = c // 2
        g = c % 2
        cols = slice(GD * g, GD * g + GD)
        im = {
            "xq": np.ascontiguousarray(query[b].T).astype(f16),
            "xk": np.ascontiguousarray(key[b].T).astype(f16),
            "xv": np.ascontiguousarray(value[b].T).astype(f16),
            "wq": np.ascontiguousarray(Wq[:, cols]).astype(f16),
            "wk": np.ascontiguousarray(Wk[:, cols]).astype(f16),
            "wv": np.ascontiguousarray(Wv[:, cols]).astype(f16),
            "wo": np.ascontiguousarray(Wo[cols, :]).astype(f16),
            "bq": bq[cols].reshape(1, GD).astype(f16),
            "bk": bk[cols].reshape(1, GD).astype(f16),
            "bv": bv[cols].reshape(1, GD).astype(f16),
            "ones_row": ones_row,
            "ones_col": ones_col,
            "ones_vcol": ones_vcol,
            "selh": selh,
            "oneh8": oneh8,
        }
        if causal:
            im["maskd"] = maskd
        else:
            im["maskt"] = maskt
        in_maps.append(im)

    res = run_bass_kernel_spmd(nc, in_maps, list(range(N_CORES)))

    out = np.empty((b_sz, s_len, d), np.float32)
    for b in range(b_sz):
        out[b] = res.results[2 * b]["out"] + res.results[2 * b + 1]["out"] + bo
    return out


# revision 31
# speedup vs baseline: 1.8555x; 1.0495x over previous
"""Multi-head causal attention (B=4, S=2048, D=1024, H=16) on 8 Trainium2
NeuronCores via Bass/Tile.

Sharding: core c handles batch b = c//2 and head-group g = c%2 (8 heads,
i.e. columns [512g, 512g+512) of Wq/Wk/Wv and rows [512g, 512g+512) of Wo).
Each core computes its 8 heads' attention and a partial output projection
[S, D]; the host sums the two head-group partials per batch and adds bo.

Matmul operands are fp16 (full-rate 1 cycle/row on the PE; fp32 accumulate
in PSUM); softmax runs in fp32 on ACT/DVE. All values are O(100) or less so
fp16 range is safe, and fp16's 10-bit mantissa keeps the end-to-end error
around 5e-4. Layouts keep every matmul at N=512 moving columns:
  qT/kT:  [dk, s]  (projection emitted transposed: lhsT=W chunk, rhs=X^T)
  v:      [s, dk]  interleaved with a ones column per head ([..v_h.., 1])
          so the attention-V matmul also produces the softmax row-sums
  scores: [sk, sq] (transposed; lhsT=kT chunk, rhs=qT) -> exp -> expT
  AV:     av[65, sq] += v_aug^T @ expT  (row 64 = softmax denominators)
  out:    partial[sq, :] = ctx^T.T @ Wo  (ctx^T is exactly the AV output)
"""

import os
import sys
import numpy as np

for _p in ("/opt/trn_rl_repo", "/root/.axon_site/_ro/trn_rl_repo"):
    if _p not in sys.path:
        sys.path.append(_p)

B, S_FULL, D, H, DK = 4, 2048, 1024, 16, 64
GD = 512          # dk span per core (8 heads)
P = 128
NPAIR = GD // P   # 4 head-pairs per core
N_CORES = 8
MASK_NEG = -8.0e9  # multiplied by the 0.125 softmax scale inside exp -> -1e9

_BUILD_CACHE = {}


def _build(s_len, causal, zero_bias):
    from contextlib import ExitStack

    import concourse.tile as tile
    from concourse import bacc, mybir

    dt = mybir.dt
    f32, f16, bf16 = dt.float32, dt.float16, dt.bfloat16
    Exp = mybir.ActivationFunctionType.Exp

    S = s_len
    SJ = S // 512     # 512-wide sq chunks
    SM = S // P       # 128-wide sk chunks
    DC = D // P       # contraction chunks for the projections

    nc = bacc.Bacc("TRN2", target_bir_lowering=False, debug=False,
                   num_devices=N_CORES)

    xq = nc.dram_tensor("xq", [D, S], f16, kind="ExternalInput")
    xk = nc.dram_tensor("xk", [D, S], f16, kind="ExternalInput")
    xv = nc.dram_tensor("xv", [D, S], f16, kind="ExternalInput")
    wq = nc.dram_tensor("wq", [D, GD], f16, kind="ExternalInput")
    wk = nc.dram_tensor("wk", [D, GD], f16, kind="ExternalInput")
    wv = nc.dram_tensor("wv", [D, GD], f16, kind="ExternalInput")
    wo = nc.dram_tensor("wo", [GD, D], f16, kind="ExternalInput")
    bq = nc.dram_tensor("bq", [1, GD], f16, kind="ExternalInput")
    bk = nc.dram_tensor("bk", [1, GD], f16, kind="ExternalInput")
    bv = nc.dram_tensor("bv", [1, GD], f16, kind="ExternalInput")
    ones_row = nc.dram_tensor("ones_row", [1, 512], f16, kind="ExternalInput")
    ones_col = nc.dram_tensor("ones_col", [1, P], f16, kind="ExternalInput")
    ones_vcol = nc.dram_tensor("ones_vcol", [P, 8], f16, kind="ExternalInput")
    # selh[:, 64r:64r+64] is row-r-one-hot: selects head r's reciprocal row
    # and broadcasts it over 64 partitions in one K=8 matmul
    selh = nc.dram_tensor("selh", [8, 8 * 64], f16, kind="ExternalInput")
    # oneh8[0, 8r:8r+8] is the one-hot row e_r: routes head r's softmax
    # denominator row into partition r of the gathered [8, 512] psum tile
    oneh8 = nc.dram_tensor("oneh8", [1, 64], f16, kind="ExternalInput")
    if causal:
        # 4 canonical diagonal-band blocks: block d, entry [p, c] masked
        # when p + 128*d > c  (value MASK_NEG, else 0)
        maskd = nc.dram_tensor("maskd", [4 * P, 512], bf16, kind="ExternalInput")
    else:
        # full transposed mask [sk, sq] * MASK_NEG
        maskt = nc.dram_tensor("maskt", [S, S], bf16, kind="ExternalInput")
    out = nc.dram_tensor("out", [S, D], f32, kind="ExternalOutput")

    with tile.TileContext(nc) as tc, ExitStack() as ctx0:
        persist = ctx0.enter_context(tc.tile_pool(name="persist", bufs=1))

        qTt = [persist.tile([P, S], f16, tag=f"qT{p}", name=f"qT{p}")
               for p in range(NPAIR)]
        kTt = [persist.tile([P, S], f16, tag=f"kT{p}", name=f"kT{p}")
               for p in range(NPAIR)]
        vt = [persist.tile([P, 8 * 65], f16, tag=f"v{m}", name=f"v{m}")
              for m in range(SM)]
        ones_row_t = persist.tile([1, 512], f16, tag="ones_row")
        ones_col_t = persist.tile([1, P], f16, tag="ones_col")
        bq_t = persist.tile([1, GD], f16, tag="bq")
        bk_t = persist.tile([1, GD], f16, tag="bk")
        bv_t = persist.tile([1, GD], f16, tag="bv")
        selh_t = persist.tile([8, 8 * 64], f16, tag="selh")
        # staged at partition 64 so its base matches asb[64:65] in the
        # denominator-gather matmul (lhsT/rhs bases must agree)
        oneh8_t = persist.tile([65, 64], f16, tag="oneh8")
        nc.sync.dma_start(selh_t[:], selh.ap())
        nc.sync.dma_start(oneh8_t[64:65, :], oneh8.ap())
        nc.sync.dma_start(ones_row_t[:], ones_row.ap())
        nc.sync.dma_start(ones_col_t[:], ones_col.ap())
        nc.sync.dma_start(bq_t[:], bq.ap())
        nc.sync.dma_start(bk_t[:], bk.ap())
        nc.sync.dma_start(bv_t[:], bv.ap())
        if causal:
            # one [128,128] block covers every diagonal mixed window:
            # within the window the pattern is always "masked iff p > c"
            mask128_t = persist.tile([P, P], bf16, tag="mask128")
            nc.sync.dma_start(mask128_t[:], maskd.ap()[0:P, 0:P])

        # ---------------- phase 1: projections ----------------
        # full-row X^T staging ([128, S] fp16); X loads dispatch from the
        # (otherwise idle) scalar engine's DGE so SP isn't a serial choke
        with ExitStack() as ctx1:
            xpool = ctx1.enter_context(tc.tile_pool(name="xt", bufs=16))
            wpool = ctx1.enter_context(tc.tile_pool(name="wt", bufs=24))
            ps1 = ctx1.enter_context(tc.tile_pool(name="ps1", bufs=3, space="PSUM"))

            # q and k projections, emitted transposed [dk, s]
            for x_d, w_d, b_t, outT in ((xq, wq, bq_t, qTt), (xk, wk, bk_t, kTt)):
                wt = []
                xt = []
                for dc in range(DC):
                    w = wpool.tile([P, GD], f16, tag="wt", name="wt")
                    nc.sync.dma_start(w[:], w_d.ap()[dc * P:(dc + 1) * P, :])
                    wt.append(w)
                    t = xpool.tile([P, S], f16, tag="xt", name="xt")
                    nc.scalar.dma_start(t[:], x_d.ap()[dc * P:(dc + 1) * P, :])
                    xt.append(t)
                for i in range(NPAIR):
                    for sj in range(SJ):
                        ps = ps1.tile([P, 512], f32, tag="ps1", name="ps1")
                        for dc in range(DC):
                            nc.tensor.matmul(
                                ps[:],
                                wt[dc][:, i * P:(i + 1) * P],
                                xt[dc][:, sj * 512:(sj + 1) * 512],
                                start=(dc == 0),
                                stop=(zero_bias and dc == DC - 1))
                        if not zero_bias:
                            nc.tensor.matmul(
                                ps[:], b_t[0:1, i * P:(i + 1) * P],
                                ones_row_t[:], start=False, stop=True)
                        nc.vector.tensor_copy(
                            outT[i][:, sj * 512:(sj + 1) * 512], ps[:])

            # v projection, natural layout [s, dk] with interleaved ones col
            wt = []
            xt = []
            for dc in range(DC):
                w = wpool.tile([P, GD], f16, tag="wt", name="wt")
                nc.sync.dma_start(w[:], wv.ap()[dc * P:(dc + 1) * P, :])
                wt.append(w)
                t = xpool.tile([P, S], f16, tag="xt", name="xt")
                nc.scalar.dma_start(t[:], xv.ap()[dc * P:(dc + 1) * P, :])
                xt.append(t)
            for si in range(SM):
                ps = ps1.tile([P, 512], f32, tag="ps1", name="ps1")
                for dc in range(DC):
                    nc.tensor.matmul(
                        ps[:],
                        xt[dc][:, si * P:(si + 1) * P],
                        wt[dc][:],
                        start=(dc == 0),
                        stop=(zero_bias and dc == DC - 1))
                if not zero_bias:
                    nc.tensor.matmul(ps[:], ones_col_t[:], bv_t[:],
                                     start=False, stop=True)
                v3 = vt[si][:].rearrange("p (h c) -> p h c", h=8)
                nc.vector.tensor_copy(
                    v3[:, :, 0:64],
                    ps[:].rearrange("p (h c) -> p h c", h=8))
                nc.sync.dma_start(v3[:, :, 64:65], ones_vcol.ap()[:, :, None])

        # ---------------- phases 2+3: attention + output projection ------
        with ExitStack() as ctx2:
            ctxpool = ctx2.enter_context(tc.tile_pool(name="ctxp", bufs=1))
            ctxt = [ctxpool.tile([P, S], f16, tag=f"ctx{p}", name=f"ctx{p}")
                    for p in range(NPAIR)]
            wopool = ctx2.enter_context(tc.tile_pool(name="wo", bufs=1))
            scps = ctx2.enter_context(tc.tile_pool(name="scps", bufs=2, space="PSUM"))
            avps = ctx2.enter_context(tc.tile_pool(name="avps", bufs=4, space="PSUM"))
            ps3 = avps
            expp = ctx2.enter_context(tc.tile_pool(name="expp", bufs=6))
            avsb = ctx2.enter_context(tc.tile_pool(name="avsb", bufs=12))
            rcp = ctx2.enter_context(tc.tile_pool(name="rcp", bufs=2))
            osb = ctx2.enter_context(tc.tile_pool(name="osb", bufs=4))
            if not causal:
                mpool = ctx2.enter_context(tc.tile_pool(name="mp", bufs=SM + 2))

            wot = [[wopool.tile([P, 512], f16, tag=f"wo{p}_{h}", name=f"wo{p}_{h}")
                    for h in range(D // 512)] for p in range(NPAIR)]
            for p in range(NPAIR):
                for h in range(D // 512):
                    nc.sync.dma_start(
                        wot[p][h][:],
                        wo.ap()[p * P:(p + 1) * P, h * 512:(h + 1) * 512])

            def emit_outproj_si(si):
                # output projection for one 128-row block of sq
                if True:
                    ot = osb.tile([P, D], f32, tag="ot", name="ot")
                    for h in range(D // 512):
                        ps = ps3.tile([P, 512], f32, tag="av", name="ps3")
                        for p in range(NPAIR):
                            nc.tensor.matmul(
                                ps[:],
                                ctxt[p][:, si * P:(si + 1) * P],
                                wot[p][h][:],
                                start=(p == 0), stop=(p == NPAIR - 1))
                        nc.vector.tensor_copy(ot[:, h * 512:(h + 1) * 512], ps[:])
                    nc.sync.dma_start(out.ap()[si * P:(si + 1) * P, :], ot[:])

            pending_si = []
            for j in range(SJ):
                n_m = 4 * (j + 1) if causal else SM
                if not causal:
                    mt = []
                    for m in range(SM):
                        t = mpool.tile([P, 512], bf16, tag="mt", name="mt")
                        nc.sync.dma_start(
                            t[:], maskt.ap()[m * P:(m + 1) * P,
                                             j * 512:(j + 1) * 512])
                        mt.append(t)
                den = avps.tile([8, 512], f32, tag="av", name="den")
                asb_all = {}
                for p in range(NPAIR):
                    # out-projection rows of the previous chunk interleave
                    # here: independent PE work that covers this chunk's
                    # exp/normalization latency
                    if pending_si:
                        emit_outproj_si(pending_si.pop(0))
                    av = [avps.tile([65, 512], f32, tag="av", name="av")
                          for _ in range(2)]
                    pend = []  # (m, exp_tile, c0) awaiting their AV matmuls
                    for m in range(n_m):
                        # causal diagonal block d: columns [0, 128d) of this
                        # sq chunk are fully masked -> compute only the
                        # suffix [c0, 512) in scores/exp/AV; the mixed
                        # 128-col window gets the shared p>c mask block
                        d = m - 4 * j if causal else -1
                        c0 = 128 * d if d > 0 else 0
                        nv = 512 - c0
                        sc = scps.tile([P, 1024], f32, tag="sc", name="sc")
                        for e in range(2):
                            nc.tensor.matmul(
                                sc[:, e * 512 + c0:(e + 1) * 512],
                                kTt[p][64 * e:64 * e + 64, m * P:(m + 1) * P],
                                qTt[p][64 * e:64 * e + 64,
                                       j * 512 + c0:(j + 1) * 512],
                                start=True, stop=True)
                        sc3 = sc[:].rearrange("p (e c) -> p e c", e=2)
                        if causal:
                            if d >= 0:
                                nc.vector.tensor_add(
                                    sc3[:, :, c0:c0 + P], sc3[:, :, c0:c0 + P],
                                    mask128_t[:][:, None, :].broadcast_to(
                                        [P, 2, P]))
                        else:
                            nc.vector.tensor_add(
                                sc3, sc3,
                                mt[m][:][:, None, :].broadcast_to([P, 2, 512]))
                        ex = expp.tile([P, 1024], f16, tag="ex", name="ex")
                        ex3 = ex[:].rearrange("p (e c) -> p e c", e=2)
                        nc.scalar.activation(ex3[:, :, c0:512],
                                             sc3[:, :, c0:512], Exp, scale=0.125)
                        pend.append((m, ex, c0))
                        if len(pend) > 2:
                            pm, pex, pc0 = pend.pop(0)
                            for e in range(2):
                                nc.tensor.matmul(
                                    av[e][:, pc0:512],
                                    vt[pm][:, 65 * (2 * p + e):65 * (2 * p + e) + 65],
                                    pex[:, e * 512 + pc0:(e + 1) * 512],
                                    start=(pm == 0), stop=(pm == n_m - 1))
                    for pm, pex, pc0 in pend:
                        for e in range(2):
                            nc.tensor.matmul(
                                av[e][:, pc0:512],
                                vt[pm][:, 65 * (2 * p + e):65 * (2 * p + e) + 65],
                                pex[:, e * 512 + pc0:(e + 1) * 512],
                                start=(pm == 0), stop=(pm == n_m - 1))
                    # stage av in SBUF; route its denominator row (base
                    # partition 64, which matmul rhs allows) into partition
                    # 2p+e of the shared den psum tile via a one-hot K=1 MM
                    for e in range(2):
                        r = 2 * p + e
                        asb = avsb.tile([65, 512], f16, tag="asb", name="asb")
                        nc.vector.tensor_copy(asb[:], av[e][:])
                        nc.tensor.matmul(den[:], oneh8_t[64:65, 8 * r:8 * r + 8],
                                         asb[64:65, :],
                                         start=(r == 0), stop=(r == 7))
                        asb_all[(p, e)] = asb

                # one batched reciprocal for all 8 heads of this j chunk
                rinv = rcp.tile([8, 512], f16, tag="rinv", name="rinv")
                with nc.allow_low_precision(
                        reason="softmax denominators are O(1..3e4); fp16 "
                               "reciprocal keeps ~5e-4 rel err"):
                    nc.vector.reciprocal(rinv[:], den[:])
                # normalize: ctx^T[dk, sq] = av[0:64] * (1/av[64]) per head
                for p in range(NPAIR):
                    for e in range(2):
                        r = 2 * p + e
                        bc = avps.tile([65, 512], f32, tag="av", name="bc")
                        nc.tensor.matmul(bc[0:64, :],
                                         selh_t[:, 64 * r:64 * r + 64],
                                         rinv[:], start=True, stop=True)
                        nc.vector.tensor_mul(
                            ctxt[p][64 * e:64 * e + 64, j * 512:(j + 1) * 512],
                            asb_all[(p, e)][0:64, :], bc[0:64, :])
                pending_si.extend(range(4 * j, 4 * j + 4))
            for si in pending_si:
                emit_outproj_si(si)

    nc.compile()
    return nc


def _get_nc(s_len, causal, zero_bias):
    key = (s_len, causal, zero_bias)
    if key not in _BUILD_CACHE:
        _BUILD_CACHE[key] = _build(s_len, causal, zero_bias)
    return _BUILD_CACHE[key]


def kernel(query, key, value, mask, Wq, bq, Wk, bk, Wv, bv, Wo, bo):
    import ml_dtypes
    from concourse.bass_utils import run_bass_kernel_spmd

    query = np.asarray(query, dtype=np.float32)
    key = np.asarray(key, dtype=np.float32)
    value = np.asarray(value, dtype=np.float32)
    mask = np.asarray(mask, dtype=np.float32)
    Wq, Wk, Wv, Wo = (np.asarray(w, dtype=np.float32) for w in (Wq, Wk, Wv, Wo))
    bq, bk, bv, bo = (np.asarray(b, dtype=np.float32) for b in (bq, bk, bv, bo))

    b_sz, s_len, d = query.shape
    m2 = mask.reshape(s_len, s_len)
    causal = bool(
        np.array_equal(m2, np.triu(np.ones((s_len, s_len), np.float32), k=1)))

    zero_bias = not (bq.any() or bk.any() or bv.any())
    nc = _get_nc(s_len, causal, zero_bias)

    f16 = np.float16
    ones_row = np.ones((1, 512), f16)
    ones_col = np.ones((1, P), f16)
    ones_vcol = np.ones((P, 8), f16)
    selh = np.zeros((8, 8 * 64), f16)
    for r in range(8):
        selh[r, 64 * r:64 * r + 64] = 1.0
    oneh8 = np.zeros((1, 64), f16)
    oneh8[0, 9 * np.arange(8)] = 1.0
    if causal:
        # maskd[d][p, c] = MASK_NEG where p + 128*d > c
        pp = np.arange(P)[:, None]
        cc = np.arange(512)[None, :]
        maskd = np.concatenate(
            [np.where(pp + P * dd > cc, MASK_NEG, 0.0) for dd in range(4)],
            axis=0).astype(ml_dtypes.bfloat16)
    else:
        maskt = (m2.T * MASK_NEG).astype(ml_dtypes.bfloat16)

    in_maps = []
    for c in range(N_CORES):
        b = c // 2
        g = c % 2
        cols = slice(GD * g, GD * g + GD)
        im = {
            "xq": np.ascontiguousarray(query[b].T).astype(f16),
            "xk": np.ascontiguousarray(key[b].T).astype(f16),
            "xv": np.ascontiguousarray(value[b].T).astype(f16),
            "wq": np.ascontiguousarray(Wq[:, cols]).astype(f16),
            "wk": np.ascontiguousarray(Wk[:, cols]).astype(f16),
            "wv": np.ascontiguousarray(Wv[:, cols]).astype(f16),
            "wo": np.ascontiguousarray(Wo[cols, :]).astype(f16),
            "bq": bq[cols].reshape(1, GD).astype(f16),
            "bk": bk[cols].reshape(1, GD).astype(f16),
            "bv": bv[cols].reshape(1, GD).astype(f16),
            "ones_row": ones_row,
            "ones_col": ones_col,
            "ones_vcol": ones_vcol,
            "selh": selh,
            "oneh8": oneh8,
        }
        if causal:
            im["maskd"] = maskd
        else:
            im["maskt"] = maskt
        in_maps.append(im)

    res = run_bass_kernel_spmd(nc, in_maps, list(range(N_CORES)))

    out = np.empty((b_sz, s_len, d), np.float32)
    for b in range(b_sz):
        out[b] = res.results[2 * b]["out"] + res.results[2 * b + 1]["out"] + bo
    return out
